# revision 41
# baseline (speedup 1.0000x reference)
"""Multi-Head Latent Attention (MLA) TRN2 Bass kernel.

Sharding: data-parallel over batch (B=2) x tensor-parallel over heads
(16 heads -> 4 per core) = 8 cores. The kv_lora latent path and shared
rope key are computed replicated within each batch group; the final
output projection is computed as per-core partials which the host sums.

All on-device dataflow is "transposed" (feature dim on partitions,
sequence on the free dim) so no PE transposes are ever needed:
  qT      = Wq_perm^T @ xT          [768, S]   (nope tiles 0..3 per head,
                                                rope tiles 4..5)
  kv_aT   = Wkv_a^T @ xT            [576, S]   (c_kvT rows 0..511,
                                                k_ropeT rows 512..575)
  k_nopeT = Wkv_b_k^T @ c_kvT       [512, S]
  v       = (c_kvT chunk)^T-matmuls [S, 512]   (natural layout)
  RoPE applied in transposed layout with a partition-swap DMA + 3 DVE ops
  scoresT[s_k, s_q] per (head, q-block of 512), causal masks added on the
  4 diagonal chunks, exp on ACT (no max subtraction; scores are bounded),
  denominators via ones-matmul accumulation, out^T accumulated in PSUM,
  normalized by broadcasted reciprocals, then partialT = Wo_c^T @ outT.
"""

import math
import sys

import numpy as np
import ml_dtypes

try:  # concourse ships in the container; fall back to the repo checkout
    import concourse.bass  # noqa: F401
except ImportError:  # pragma: no cover
    for p in ("/opt/trn_rl_repo", "/root/.axon_site/_ro/trn_rl_repo"):
        if p not in sys.path:
            sys.path.insert(0, p)

# Problem constants (hardcoded; harness calls kernel() standalone).
D_MODEL = 2048
N_HEADS = 16
R = 512          # kv lora rank
DN = 128         # d_nope
DR = 64          # d_rope
DV = 128         # d_v
ROPE_THETA = 10000.0
B = 2
S = 2048
HP = 4           # heads per core
QB = 512         # q block size
NKC = S // 128   # 16 k chunks
NQB = S // QB    # 4 q blocks
NCORES = 8

BF16 = ml_dtypes.bfloat16

_PROGRAM = {}


def _build_program(split_waits=True):
    import concourse.bass as bass
    import concourse.mybir as mybir
    from concourse.tile import TileContext

    def split_multi_waits(max_waits=1):
        """The walrus build in this container rejects instructions with
        more than `max_waits` sync-wait commands. Move excess waits onto
        same-engine NoOps inserted just before the instruction."""
        for f in nc.m.functions:
            for bb in f.blocks:
                out = []
                changed = False
                for inst in bb.instructions:
                    si = getattr(inst, "sync_info", None)
                    ws = list(si.on_wait) if si is not None else []
                    if len(ws) > max_waits:
                        changed = True
                        inst.sync_info = mybir.SyncInfo(
                            on_wait=ws[:max_waits],
                            on_update=list(si.on_update))
                        for w in ws[max_waits:]:
                            n = mybir.InstNoOp(
                                name=nc.get_next_instruction_name(),
                                ins=[], outs=[])
                            n.engine = inst.engine
                            n.sync_info = mybir.SyncInfo(
                                on_wait=[w], on_update=[])
                            out.append(n)
                    out.append(inst)
                if changed:
                    bb.instructions = out

    f32 = mybir.dt.float32
    cdt = mybir.dt.bfloat16

    nc = bass.Bass()

    xT = nc.dram_tensor("xT", [D_MODEL, S], cdt, kind="ExternalInput")
    wq = nc.dram_tensor("wq", [D_MODEL, HP * (DN + DR)], cdt, kind="ExternalInput")
    wkva = nc.dram_tensor("wkva", [D_MODEL, R + DR], cdt, kind="ExternalInput")
    wkvbk = nc.dram_tensor("wkvbk", [R, HP * DN], cdt, kind="ExternalInput")
    wkvbv = nc.dram_tensor("wkvbv", [R, HP * DV], cdt, kind="ExternalInput")
    wo = nc.dram_tensor("wo", [HP * DV, D_MODEL], cdt, kind="ExternalInput")
    cosf = nc.dram_tensor("cosf", [128, S], cdt, kind="ExternalInput")
    sinf = nc.dram_tensor("sinf", [128, S], cdt, kind="ExternalInput")
    masks = nc.dram_tensor("masks", [128, 128], cdt, kind="ExternalInput")
    ident = nc.dram_tensor("ident", [128, 128], cdt, kind="ExternalInput")
    ones = nc.dram_tensor("ones", [128, 1], cdt, kind="ExternalInput")
    onesf = nc.dram_tensor("onesf", [1, 128], cdt, kind="ExternalInput")
    outp = nc.dram_tensor("outp", [D_MODEL, S], f32, kind="ExternalOutput")

    Exp = mybir.ActivationFunctionType.Exp
    Ln = mybir.ActivationFunctionType.Ln

    with TileContext(nc) as tc:
        with (
            tc.tile_pool(name="const", bufs=1) as cpool,
            tc.tile_pool(name="persist", bufs=1) as ppool,
        ):
            cosf_sb = cpool.tile([128, S], cdt, name="cosf_sb")
            sinf_sb = cpool.tile([128, S], cdt, name="sinf_sb")
            masks_sb = cpool.tile([128, 128], cdt, name="masks_sb")
            ident_sb = cpool.tile([128, 128], cdt, name="ident_sb")
            ones_sb = cpool.tile([128, 1], cdt, name="ones_sb")
            onesb_sb = cpool.tile([1, 128], cdt, name="onesb_sb")

            # Persistent activations.
            qT = [
                ppool.tile([128, S], cdt, name=f"qT{m}", tag="qT", bufs=6)
                for m in range(6)
            ]
            ck = [
                ppool.tile([128, S], cdt, name=f"ck{m}", tag="cko", bufs=4)
                for m in range(4)
            ]
            kn = [
                ppool.tile([128, S], cdt, name=f"kn{m}", tag="kn", bufs=4)
                for m in range(4)
            ]
            kr = ppool.tile([128, S], cdt, name="krope", tag="krope", bufs=1)
            vt = [
                ppool.tile([128, HP * DV], cdt, name=f"v{i}", tag="v", bufs=NKC)
                for i in range(NKC)
            ]
            # RoPE swap scratch lives in the persistent pool so the kvT
            # weight pool does not WAR-serialize against the rope phase.
            swt = [
                ppool.tile([128, S], cdt, name=f"sw{i}", tag="sw", bufs=3)
                for i in range(3)
            ]

            # kv_b weights persist so their DMAs can issue at startup.
            wbk_sb = [
                ppool.tile([128, HP * DN], cdt, name=f"wbk_sb{r}", tag="wbk",
                           bufs=4)
                for r in range(4)
            ]
            wbv_sb = [
                ppool.tile([128, HP * DV], cdt, name=f"wbv_sb{r}", tag="wbv",
                           bufs=4)
                for r in range(4)
            ]

            # ---- Phase 1: x projections, then kv up-projection ----
            with (
                tc.tile_pool(name="wproj", bufs=1) as wpool,
                tc.tile_pool(name="xstream", bufs=1) as xpool,
                tc.tile_pool(name="psA", bufs=8, space="PSUM") as psA,
            ):
                # Interleave weight-chunk and first-quarter x DMAs across two
                # HWDGE queues so the first matmuls can start early.
                wq_sb = []
                wkva_sb = []
                xq0 = []
                for k in range(16):
                    xk = xpool.tile([128, QB], cdt, name=f"xq_0_{k}",
                                    tag="xq", bufs=24)
                    nc.sync.dma_start(xk, xT[k * 128:(k + 1) * 128, 0:QB])
                    xq0.append(xk)
                    w1 = wpool.tile([128, HP * (DN + DR)], cdt, name=f"wq_sb{k}",
                                    tag="wq", bufs=16)
                    nc.scalar.dma_start(w1, wq[k * 128:(k + 1) * 128, :])
                    wq_sb.append(w1)
                    w2 = wpool.tile([128, R + DR], cdt, name=f"wkva_sb{k}",
                                    tag="wkva", bufs=16)
                    nc.scalar.dma_start(w2, wkva[k * 128:(k + 1) * 128, :])
                    wkva_sb.append(w2)
                for r in range(4):
                    nc.scalar.dma_start(wbk_sb[r], wkvbk[r * 128:(r + 1) * 128, :])
                    nc.scalar.dma_start(wbv_sb[r], wkvbv[r * 128:(r + 1) * 128, :])
                nc.scalar.dma_start(cosf_sb, cosf[:, :])
                nc.scalar.dma_start(sinf_sb, sinf[:, :])
                nc.scalar.dma_start(masks_sb, masks[:, :])
                nc.scalar.dma_start(ident_sb, ident[:, :])
                nc.scalar.dma_start(ones_sb, ones[:, :])
                nc.scalar.dma_start(onesb_sb, onesf[:, :])

                # Quarter 0 is DMA-latency bound: run k OUTER over the first
                # 8 M-tiles (all 8 PSUM banks) so each arriving x chunk gets
                # 8 matmuls of work instead of 1.
                t0 = slice(0, QB)
                ps8 = [
                    psA.tile([128, QB], f32, name=f"psq0_{m}", tag="ps")
                    for m in range(8)
                ]
                for k in range(16):
                    for m in range(6):
                        nc.tensor.matmul(
                            ps8[m], lhsT=wq_sb[k][:, m * 128:(m + 1) * 128],
                            rhs=xq0[k], start=(k == 0), stop=(k == 15))
                    for m in range(2):
                        nc.tensor.matmul(
                            ps8[6 + m], lhsT=wkva_sb[k][:, m * 128:(m + 1) * 128],
                            rhs=xq0[k], start=(k == 0), stop=(k == 15))
                for m in range(6):
                    nc.scalar.copy(qT[m][:, t0], ps8[m])
                for m in range(2):
                    nc.vector.tensor_copy(ck[m][:, t0], ps8[6 + m])
                for m in (2, 3):
                    ps = psA.tile([128, QB], f32, name=f"psk_0_{m}", tag="ps")
                    for k in range(16):
                        nc.tensor.matmul(
                            ps, lhsT=wkva_sb[k][:, m * 128:(m + 1) * 128],
                            rhs=xq0[k], start=(k == 0), stop=(k == 15))
                    nc.vector.tensor_copy(ck[m][:, t0], ps)
                ps = psA.tile([64, QB], f32, name="psr_0", tag="ps")
                for k in range(16):
                    nc.tensor.matmul(
                        ps, lhsT=wkva_sb[k][:, R:R + DR],
                        rhs=xq0[k], start=(k == 0), stop=(k == 15))
                nc.scalar.copy(kr[0:64, t0], ps)
                nc.scalar.copy(kr[64:128, t0], ps)

                for t in range(1, NQB):
                    tcols = slice(t * QB, (t + 1) * QB)
                    xq = []
                    for k in range(16):
                        xk = xpool.tile([128, QB], cdt, name=f"xq_{t}_{k}",
                                        tag="xq", bufs=24)
                        nc.sync.dma_start(
                            xk, xT[k * 128:(k + 1) * 128, tcols])
                        xq.append(xk)
                    # qT M-tiles
                    for m in range(6):
                        ps = psA.tile([128, QB], f32, name=f"psq_{t}_{m}", tag="ps")
                        for k in range(16):
                            nc.tensor.matmul(
                                ps, lhsT=wq_sb[k][:, m * 128:(m + 1) * 128],
                                rhs=xq[k], start=(k == 0), stop=(k == 15))
                        nc.scalar.copy(qT[m][:, tcols], ps)
                    # c_kvT M-tiles
                    for m in range(4):
                        ps = psA.tile([128, QB], f32, name=f"psk_{t}_{m}", tag="ps")
                        for k in range(16):
                            nc.tensor.matmul(
                                ps, lhsT=wkva_sb[k][:, m * 128:(m + 1) * 128],
                                rhs=xq[k], start=(k == 0), stop=(k == 15))
                        nc.vector.tensor_copy(ck[m][:, tcols], ps)
                    # k_ropeT (rows 512..575 of kv_aT), duplicated into kr
                    ps = psA.tile([64, QB], f32, name=f"psr_{t}", tag="ps")
                    for k in range(16):
                        nc.tensor.matmul(
                            ps, lhsT=wkva_sb[k][:, R:R + DR],
                            rhs=xq[k], start=(k == 0), stop=(k == 15))
                    nc.scalar.copy(kr[0:64, tcols], ps)
                    nc.scalar.copy(kr[64:128, tcols], ps)

                # ---- kv up-projection (k_nopeT, v); same PSUM pool ----
                for m in range(4):
                    for nb in range(NQB):
                        ncols = slice(nb * QB, (nb + 1) * QB)
                        ps = psA.tile([128, QB], f32, name=f"psn_{m}_{nb}",
                                      tag="ps")
                        for r in range(4):
                            nc.tensor.matmul(
                                ps, lhsT=wbk_sb[r][:, m * 128:(m + 1) * 128],
                                rhs=ck[r][:, ncols], start=(r == 0),
                                stop=(r == 3))
                        nc.scalar.copy(kn[m][:, ncols], ps)
                for i in range(NKC):
                    ps = psA.tile([128, HP * DV], f32, name=f"psv_{i}", tag="ps")
                    for r in range(4):
                        nc.tensor.matmul(
                            ps, lhsT=ck[r][:, i * 128:(i + 1) * 128],
                            rhs=wbv_sb[r], start=(r == 0), stop=(r == 3))
                    nc.vector.tensor_copy(vt[i], ps)

            # ---- Phase 2: RoPE rotation (in place; overlaps on DVE) ----
            # rot = x * cosf + swap32(x) * sinf, where swap32 swaps each
            # 32-row half within every 64-row group (signs folded in sinf).
            for idx, tap in enumerate([qT[4], qT[5], kr]):
                sw = swt[idx]
                for blk in range(4):
                    src = (blk ^ 1) * 32
                    nc.sync.dma_start(
                        sw[blk * 32:(blk + 1) * 32, :],
                        tap[src:src + 32, :])
                nc.vector.tensor_mul(tap, tap, cosf_sb)
                nc.vector.tensor_mul(sw, sw, sinf_sb)
                nc.vector.tensor_add(tap, tap, sw)

            # outT tiles reuse the c_kvT slots (same tag, 4 bufs).
            outT = [
                ppool.tile([128, S], cdt, name=f"outT{h}", tag="cko", bufs=4)
                for h in range(HP)
            ]

            # ---- Phase 3: attention + output projection ----
            with (
                tc.tile_pool(name="att", bufs=1) as apool,
                tc.tile_pool(name="psS", bufs=4, space="PSUM") as psS,
                tc.tile_pool(name="psO", bufs=2, space="PSUM") as psO,
                tc.tile_pool(name="psD", bufs=1, space="PSUM") as psD,
                tc.tile_pool(name="psBC", bufs=1, space="PSUM") as psBC,
            ):
                # Wo loads overlap the attention phase on the idle sync queue.
                wo_sb = [
                    apool.tile([128, D_MODEL], cdt, name=f"wo_sb{r}", tag="wo",
                               bufs=4)
                    for r in range(4)
                ]
                for r in range(4):
                    nc.sync.dma_start(wo_sb[r], wo[r * 128:(r + 1) * 128, :])

                def norm_early(h, j, dps):
                    # 1/denom as exp(-ln(d)) on the ACT engine: much lower
                    # latency than the DVE reciprocal and off the DVE queue.
                    # Denominators are sums of exps, comfortably inside the
                    # Ln/Exp range. The exp writes bf16 directly.
                    rec = apool.tile([1, QB], f32, name=f"rec_{h}_{j}",
                                     tag="rec", bufs=2)
                    nc.scalar.activation(rec, dps, Ln)
                    recb = apool.tile([1, QB], cdt, name=f"recb_{h}_{j}",
                                      tag="recb", bufs=2)
                    nc.scalar.activation(recb, rec, Exp, scale=-1.0)
                    return recb

                def norm_late(h, j, ops, recb):
                    # Broadcast 1/denom across partitions via a K=1 matmul,
                    # then scale the out accumulator into outT.
                    qs = slice(j * QB, (j + 1) * QB)
                    bps = psBC.tile([128, QB], f32, name=f"bps_{h}_{j}",
                                    tag="b")
                    nc.tensor.matmul(bps, lhsT=onesb_sb, rhs=recb,
                                     start=True, stop=True)
                    bc = apool.tile([128, QB], f32, name=f"bc_{h}_{j}",
                                    tag="bc", bufs=2)
                    nc.scalar.copy(bc, bps)
                    nc.vector.tensor_mul(outT[h][:, qs], ops, bc)

                pending = None
                for h in range(HP):
                    qn = qT[h]
                    qr = qT[4 + h // 2]
                    off = (h % 2) * 64
                    for j in range(NQB):
                        qs = slice(j * QB, (j + 1) * QB)
                        ops = psO.tile([128, QB], f32, name=f"ops_{h}_{j}",
                                       tag="o")
                        dps = psD.tile([1, QB], f32, name=f"dps_{h}_{j}",
                                       tag="d")
                        nch = 4 * (j + 1)
                        for c in range(nch):
                            ks = slice(c * 128, (c + 1) * 128)
                            r = c - 4 * j
                            # Diagonal chunks only need columns >= r*128
                            # (everything to the left is strictly above the
                            # causal boundary). Chunk 0 always start-covers
                            # the full accumulator width.
                            col0 = max(0, r * 128)
                            w = slice(col0, QB)
                            qsw = slice(j * QB + col0, (j + 1) * QB)
                            sps = psS.tile([128, QB], f32,
                                           name=f"sps_{h}_{j}_{c}", tag="s")
                            nc.tensor.matmul(sps[:, w], lhsT=kn[h][:, ks],
                                             rhs=qn[:, qsw], start=True,
                                             stop=False,
                                             skip_group_check=True)
                            nc.tensor.matmul(sps[:, w],
                                             lhsT=kr[off:off + 64, ks],
                                             rhs=qr[off:off + 64, qsw],
                                             start=False, stop=(r < 0),
                                             skip_group_check=True)
                            if r >= 0:
                                # Add the causal tri mask on the PE itself
                                # (identity @ tri) so exp never waits on a
                                # cross-engine DVE hop.
                                nc.tensor.matmul(
                                    sps[:, col0:col0 + 128], lhsT=ident_sb,
                                    rhs=masks_sb, start=False, stop=True,
                                    skip_group_check=True)
                            pt = apool.tile([128, QB], cdt,
                                            name=f"pt_{h}_{j}_{c}", tag="pt",
                                            bufs=6)
                            nc.scalar.activation(pt[:, w], sps[:, w], Exp)
                            nc.tensor.matmul(
                                ops[:, w], lhsT=vt[c][:, h * DV:(h + 1) * DV],
                                rhs=pt[:, w], start=(c == 0),
                                stop=(c == nch - 1), skip_group_check=True)
                            nc.tensor.matmul(
                                dps[0:1, w], lhsT=ones_sb, rhs=pt[:, w],
                                start=(c == 0), stop=(c == nch - 1),
                                skip_group_check=True)
                            if c == 3 and pending is not None:
                                norm_late(*pending)
                                pending = None
                        recb = norm_early(h, j, dps)
                        pending = (h, j, ops, recb)
                norm_late(*pending)

                # ---- Output projection; PSUM reuses the score slots ----
                for m in range(16):
                    st = apool.tile([128, S], f32, name=f"st_{m}", tag="st",
                                    bufs=2)
                    for nb in range(NQB):
                        ncols = slice(nb * QB, (nb + 1) * QB)
                        # Alternate between the (now idle) score and out
                        # accumulator slots for a deeper pipeline.
                        wopool, wotag = (psS, "s") if nb % 2 == 0 else (psO, "o")
                        ps = wopool.tile([128, QB], f32, name=f"psw_{m}_{nb}",
                                         tag=wotag)
                        for r in range(4):
                            nc.tensor.matmul(
                                ps, lhsT=wo_sb[r][:, m * 128:(m + 1) * 128],
                                rhs=outT[r][:, ncols], start=(r == 0),
                                stop=(r == 3))
                        if nb % 2 == 0:
                            nc.scalar.copy(st[:, ncols], ps)
                        else:
                            nc.vector.tensor_copy(st[:, ncols], ps)
                    nc.sync.dma_start(outp[m * 128:(m + 1) * 128, :], st)

    if split_waits:
        split_multi_waits()
    return nc


def get_program(split_waits=True):
    if split_waits not in _PROGRAM:
        _PROGRAM[split_waits] = _build_program(split_waits)
    return _PROGRAM[split_waits]


def make_core_inputs(x, Wq, Wkv_a, Wkv_b, Wo):
    """Host-side sharding/pre-processing. Returns list of 8 input dicts."""
    scale = 1.0 / math.sqrt(DN + DR)

    inv_freq = 1.0 / (ROPE_THETA ** (np.arange(0, DR, 2, dtype=np.float64) / DR))
    t = np.arange(S, dtype=np.float64)
    freqs = np.outer(t, inv_freq)                      # [S, 32]
    cos32 = np.cos(freqs).T.astype(np.float32)         # [32, S]
    sin32 = np.sin(freqs).T.astype(np.float32)
    cosf = np.tile(cos32, (4, 1)).astype(BF16)         # [128, S]
    sinf = np.tile(np.concatenate([-sin32, sin32], axis=0), (2, 1)).astype(BF16)

    row = np.arange(128)[:, None]
    col = np.arange(128)[None, :]
    masks = np.where(col >= row, 0.0, -1e30).astype(BF16)  # [128, 128]
    ident = np.eye(128, dtype=BF16)
    ones = np.ones([128, 1], dtype=BF16)
    onesf = np.ones([1, 128], dtype=BF16)

    Wq_r = np.asarray(Wq, dtype=np.float32).reshape(D_MODEL, N_HEADS, DN + DR)
    Wb_r = np.asarray(Wkv_b, dtype=np.float32).reshape(R, N_HEADS, DN + DV)
    Wo_f = np.asarray(Wo, dtype=np.float32)
    Wkva_f = np.asarray(Wkv_a, dtype=np.float32).astype(BF16)
    x_f = np.asarray(x, dtype=np.float32)

    in_maps = []
    for c in range(NCORES):
        b, g = divmod(c, HP)
        heads = list(range(HP * g, HP * g + HP))
        xTc = np.ascontiguousarray(x_f[b].T).astype(BF16)
        wq_nope = Wq_r[:, heads, :DN].reshape(D_MODEL, HP * DN)
        wq_rope = Wq_r[:, heads, DN:].reshape(D_MODEL, HP * DR)
        wq_c = (np.concatenate([wq_nope, wq_rope], axis=1) * scale).astype(BF16)
        wbk_c = np.ascontiguousarray(
            Wb_r[:, heads, :DN].reshape(R, HP * DN)).astype(BF16)
        wbv_c = np.ascontiguousarray(
            Wb_r[:, heads, DN:].reshape(R, HP * DV)).astype(BF16)
        wo_c = np.ascontiguousarray(
            Wo_f[HP * g * DV:(HP * g + HP) * DV, :]).astype(BF16)
        in_maps.append({
            "xT": xTc,
            "wq": np.ascontiguousarray(wq_c),
            "wkva": Wkva_f,
            "wkvbk": wbk_c,
            "wkvbv": wbv_c,
            "wo": wo_c,
            "cosf": cosf,
            "sinf": sinf,
            "masks": masks,
            "ident": ident,
            "ones": ones,
            "onesf": onesf,
        })
    return in_maps


def gather_output(results):
    """results: list of 8 dicts with 'outp' [D_MODEL, S] partials."""
    out = np.empty((B, S, D_MODEL), dtype=np.float32)
    for b in range(B):
        acc = results[HP * b]["outp"].astype(np.float32).copy()
        for g in range(1, HP):
            acc += results[HP * b + g]["outp"]
        out[b] = acc.T
    return out


def kernel(x, Wq, Wkv_a, Wkv_b, Wo):
    from concourse.bass_utils import run_bass_kernel_spmd

    nc = get_program()
    in_maps = make_core_inputs(x, Wq, Wkv_a, Wkv_b, Wo)
    res = run_bass_kernel_spmd(nc, in_maps, list(range(NCORES)))
    return gather_output(res.results)
